# revision 3
# baseline (speedup 1.0000x reference)
"""Trainium2 Bass kernel for a 2-layer tanh DeepRNN.

Problem: inputs [64, 1024, 256] fp32, two stacked RNN layers (H=512):
    h0_t = tanh(x_t @ W_xh0 + h0_{t-1} @ W_hh0 + b_h0)
    h1_t = tanh(h0_t @ W_xh1 + h1_{t-1} @ W_hh1 + b_h1)
Output: h1 sequence [64, 1024, 512] fp32.

Sharding: data-parallel over batch, 8 cores x B_local=8, weights replicated.

Single-core schedule (v2): the two layer recurrences are *independent
dependency chains* (layer 1 runs 2 chunks behind layer 0), so their per-step
instructions are interleaved on the PE queue at step granularity — while
layer 0's tanh round-trips through the ACT engine, the PE runs layer 1's
matmuls for its own step, and vice versa.  tanh is issued per 128-row output
chunk (mc) straight after that chunk's 4 contraction matmuls, with the bias
applied by the ACT engine's bias port (no bias matmuls).  The batched input
projections / x transposes / DMAs for neighbouring chunks are sprinkled
between steps as filler so they never form a serial bubble.  bf16 weights
with fp32 PSUM accumulation.
"""

import sys

import numpy as np

sys.path.insert(0, "/opt/trn_rl_repo")

import ml_dtypes  # noqa: E402

import concourse.bacc as bacc  # noqa: E402
import concourse.tile as tile  # noqa: E402
from concourse import mybir  # noqa: E402
from concourse.bass_utils import run_bass_kernel_spmd  # noqa: E402

F32 = mybir.dt.float32
BF16 = mybir.dt.bfloat16
Tanh = mybir.ActivationFunctionType.Tanh

B_FULL, T, I, H = 64, 1024, 256, 512
NCORES = 8
B = B_FULL // NCORES  # 8 rows per core
CH = 16               # timesteps per chunk (16*8 cols per mc = one PSUM bank)
NCH = T // CH
S0 = 64               # h0T ring slots (4 chunks: consumed up to 2 chunks late)
S1 = 32               # h1T ring slots


def _mm(nc, out, lhsT, rhs, start, stop):
    nc.tensor.matmul(out, lhsT, rhs, start=start, stop=stop, skip_group_check=True)


def build_nc(nch=NCH, reps=1):
    nc = bacc.Bacc("TRN2", target_bir_lowering=False, debug=False)

    x_d = nc.dram_tensor("x", [B, T, I], F32, kind="ExternalInput")
    wxh0_d = nc.dram_tensor("W_xh0", [I, H], F32, kind="ExternalInput")
    whh0_d = nc.dram_tensor("W_hh0", [H, H], F32, kind="ExternalInput")
    b0_d = nc.dram_tensor("b_h0", [H], F32, kind="ExternalInput")
    wxh1_d = nc.dram_tensor("W_xh1", [H, H], F32, kind="ExternalInput")
    whh1_d = nc.dram_tensor("W_hh1", [H, H], F32, kind="ExternalInput")
    b1_d = nc.dram_tensor("b_h1", [H], F32, kind="ExternalInput")
    ident_d = nc.dram_tensor("ident", [128, 128], BF16, kind="ExternalInput")
    out_d = nc.dram_tensor("out", [B, T, H], F32, kind="ExternalOutput")

    with tile.TileContext(nc) as tc:
        _body(tc, nch, x_d, (wxh0_d, whh0_d, b0_d, wxh1_d, whh1_d, b1_d),
              ident_d, out_d, reps=reps)
    nc.compile()
    return nc


def _body(tc, nch, x_d, w_d, ident_d, out_d, reps=1):
    import contextlib

    nc = tc.nc
    wxh0_d, whh0_d, b0_d, wxh1_d, whh1_d, b1_d = w_d

    ctx = contextlib.ExitStack()
    with ctx:
        consts = ctx.enter_context(tc.tile_pool(name="consts", bufs=1))
        wstage = ctx.enter_context(tc.tile_pool(name="wstage", bufs=2))
        xpool = ctx.enter_context(tc.tile_pool(name="xpool", bufs=4))
        state = ctx.enter_context(tc.tile_pool(name="state", bufs=1))
        stg = ctx.enter_context(tc.tile_pool(name="stg", bufs=4))
        ps_l0 = ctx.enter_context(tc.tile_pool(name="ps_l0", bufs=2, space="PSUM"))
        ps_l1 = ctx.enter_context(tc.tile_pool(name="ps_l1", bufs=2, space="PSUM"))
        ps_xt = ctx.enter_context(tc.tile_pool(name="ps_xt", bufs=2, space="PSUM"))
        ps_ot = ctx.enter_context(tc.tile_pool(name="ps_ot", bufs=2, space="PSUM"))

        # ---- one-time constants: identity, weights (fp32 -> bf16) ----
        ident = consts.tile([128, 128], BF16, tag="ident")
        nc.sync.dma_start(ident[:], ident_d.ap()[:])

        def load_w(dram_ap, rows, name):
            # one [128,128] tile per (kc, mc) chunk so every matmul's
            # stationary operand is a whole tile at offset 0 -- keeps the
            # compiler's fast-weight-load eligibility unambiguous
            tiles = []
            for kc in range(rows // 128):
                tmp = wstage.tile([128, H], F32, tag="wtmp")
                nc.sync.dma_start(tmp[:], dram_ap[kc * 128:(kc + 1) * 128, :])
                row = []
                for mc in range(4):
                    wt = consts.tile([128, 128], BF16, tag=f"{name}_{kc}_{mc}")
                    nc.vector.tensor_copy(wt[:], tmp[:, mc * 128:(mc + 1) * 128])
                    row.append(wt)
                tiles.append(row)
            return tiles

        wxh0 = load_w(wxh0_d.ap(), I, "wxh0")   # [ic][hc] tiles
        whh0 = load_w(whh0_d.ap(), H, "whh0")   # [kc][mc]
        wxh1 = load_w(wxh1_d.ap(), H, "wxh1")
        whh1 = load_w(whh1_d.ap(), H, "whh1")

        def load_b(dram_ap, name):
            # [128, 4] f32: partition = h within chunk, free = mc
            bt = consts.tile([128, 4], F32, tag=name)
            nc.sync.dma_start(bt[:], dram_ap.rearrange("(m p) -> p m", p=128))
            return bt

        b0T = load_b(b0_d.ap(), "b0T")
        b1T = load_b(b1_d.ap(), "b1T")

        # ---- recurrent state rings: slots of [128, 32] (free = mc*8+b) ----
        h0T = state.tile([128, S0 * 32], BF16, tag="h0T")
        h1T = state.tile([128, S1 * 32], BF16, tag="h1T")
        nc.gpsimd.memset(h0T[:, (S0 - 1) * 32:S0 * 32], 0.0)  # h_{-1} = 0
        nc.gpsimd.memset(h1T[:, (S1 - 1) * 32:S1 * 32], 0.0)

        x_src = x_d.ap().rearrange("b (c t) i -> c t b i", t=CH)       # [64,16,8,256]
        out_dst = out_d.ap().rearrange(
            "b (c g t) (hc p) -> c g t hc b p", g=CH // 4, t=4, p=128)
        h0Tv = h0T[:].rearrange("p (s m b) -> p s m b", s=S0, b=B)

        ps0_t = {}
        ps1_t = {}
        x_t = {}

        # ---------- filler ops (emitted between recurrence steps) ----------
        def f_dma_x(c):
            def run():
                xf = xpool.tile([128, I], F32, tag="xf")
                nc.sync.dma_start(xf[:], x_src[c])
                x_t[c] = xf
            return run

        def f_xT(c):
            # x chunk -> bf16 -> PE-transpose -> xT [128(i), 2ic*128(t,b)]
            def conv():
                xb = xpool.tile([128, I], BF16, tag="xb")
                nc.vector.tensor_copy(xb[:], x_t.pop(c)[:])
                x_t[(c, "b")] = xb
            def tp():
                xb = x_t.pop((c, "b"))
                xtp = ps_xt.tile([128, I], BF16, tag="xtp")
                for ic in range(2):
                    nc.tensor.matmul(xtp[:, ic * 128:(ic + 1) * 128],
                                     xb[:, ic * 128:(ic + 1) * 128], ident[:],
                                     is_transpose=True, start=(ic == 0),
                                     stop=(ic == 1), skip_group_check=True)
                x_t[(c, "p")] = xtp
            def back():
                xT = xpool.tile([128, I], BF16, tag="xT")
                nc.vector.tensor_copy(xT[:], x_t.pop((c, "p"))[:])
                x_t[(c, "T")] = xT
            return [conv, tp, back]

        def f_xproj(c):
            # batched input projection for l0 chunk c into a fresh psum tile
            def mk():
                ps0_t[c] = ps_l0.tile([128, 512], F32, tag="ps0", name="ps0")
            ops = [mk]
            def mmop(hc, ic):
                def run():
                    xT = x_t[(c, "T")]
                    _mm(nc, ps0_t[c][:, hc * 128:(hc + 1) * 128],
                        wxh0[ic][hc][:], xT[:, ic * 128:(ic + 1) * 128],
                        start=(hc == 0 and ic == 0), stop=False)
                return run
            for hc in range(4):
                for ic in range(2):
                    ops.append(mmop(hc, ic))
            def done():
                x_t.pop((c, "T"), None)
            ops.append(done)
            return ops

        def f_l1proj(c):
            # batched h0 projection for l1 chunk c into a fresh psum tile
            base = (c % 4) * CH
            def mk():
                ps1_t[c] = ps_l1.tile([128, 512], F32, tag="ps1", name="ps1")
            ops = [mk]
            def mmop(hc, kc):
                def run():
                    _mm(nc, ps1_t[c][:, hc * 128:(hc + 1) * 128],
                        wxh1[kc][hc][:], h0Tv[:, base:base + CH, kc],
                        start=(hc == 0 and kc == 0), stop=False)
                return run
            for hc in range(4):
                for kc in range(4):
                    ops.append(mmop(hc, kc))
            return ops

        # ---------- recurrence steps ----------
        def l0_step(c, tt):
            ps = ps0_t[c]
            s = (c % 4) * CH + tt
            prev = ((s - 1) % S0) * 32
            for mc in range(4):
                o = mc * 128 + tt * 8
                for kc in range(4):
                    _mm(nc, ps[:, o:o + 8], whh0[kc][mc][:],
                        h0T[:, prev + kc * 8:prev + kc * 8 + 8],
                        start=False, stop=(kc == 3))
                nc.scalar.activation(h0T[:, s * 32 + mc * 8:s * 32 + mc * 8 + 8],
                                     ps[:, o:o + 8], Tanh,
                                     bias=b0T[:, mc:mc + 1])
            if tt == CH - 1:
                ps0_t.pop(c)

        def l1_step(c, tt):
            ps = ps1_t[c]
            s = (c % 2) * CH + tt
            prev = ((s - 1) % S1) * 32
            for mc in range(4):
                o = mc * 128 + tt * 8
                for kc in range(4):
                    _mm(nc, ps[:, o:o + 8], whh1[kc][mc][:],
                        h1T[:, prev + kc * 8:prev + kc * 8 + 8],
                        start=False, stop=(kc == 3))
                nc.scalar.activation(h1T[:, s * 32 + mc * 8:s * 32 + mc * 8 + 8],
                                     ps[:, o:o + 8], Tanh,
                                     bias=b1T[:, mc:mc + 1])
            if tt % 4 == 3:
                g = tt // 4
                tp = ps_ot.tile([128, 128], BF16, tag="otp")
                nc.tensor.transpose(
                    tp[:], h1T[:, (s - 3) * 32:(s + 1) * 32], ident[:])
                st = stg.tile([128, 128], F32, tag="ost")
                nc.vector.tensor_copy(st[:], tp[:])
                nc.sync.dma_start(out_dst[c, g], st[:])
            if tt == CH - 1:
                ps1_t.pop(c)

        def main_loop():
            # prologue: get chunk 0 (and chunk 1's x) in flight
            for op in [f_dma_x(0), f_dma_x(1)] + f_xT(0) + f_xproj(0):
                op()
            for c in range(nch + 2):
                fillers = []
                if c + 2 < nch:
                    fillers.append(f_dma_x(c + 2))
                if c + 1 < nch:
                    fillers += f_xT(c + 1) + f_xproj(c + 1)
                if 0 <= c - 1 < nch:
                    fillers += f_l1proj(c - 1)
                nf = len(fillers)
                fi = 0
                for tt in range(CH):
                    if c < nch:
                        l0_step(c, tt)
                    if c >= 2:
                        l1_step(c - 2, tt)
                    tgt = (nf * (tt + 1)) // CH
                    while fi < tgt:
                        fillers[fi]()
                        fi += 1

        if reps > 1:
            # timing mode: repeat the whole body on-device so the kernel time
            # dominates the (network-tunneled) host<->device transfer wall.
            with tc.For_i(0, reps, 1):
                main_loop()
        else:
            main_loop()


_NC_CACHE = {}


def _get_nc(nch=NCH):
    if nch not in _NC_CACHE:
        _NC_CACHE[nch] = build_nc(nch)
    return _NC_CACHE[nch]


def kernel(**inputs):
    x = np.asarray(inputs["inputs"], dtype=np.float32)
    ident = np.eye(128, dtype=ml_dtypes.bfloat16)
    shared = {
        "W_xh0": np.asarray(inputs["W_xh0"], np.float32),
        "W_hh0": np.asarray(inputs["W_hh0"], np.float32),
        "b_h0": np.asarray(inputs["b_h0"], np.float32),
        "W_xh1": np.asarray(inputs["W_xh1"], np.float32),
        "W_hh1": np.asarray(inputs["W_hh1"], np.float32),
        "b_h1": np.asarray(inputs["b_h1"], np.float32),
        "ident": ident,
    }
    in_maps = [dict(shared, x=np.ascontiguousarray(x[c * B:(c + 1) * B]))
               for c in range(NCORES)]
    nc = _get_nc()
    res = run_bass_kernel_spmd(nc, in_maps, core_ids=list(range(NCORES)))
    return np.concatenate([r["out"] for r in res.results], axis=0)


# revision 5
# speedup vs baseline: 1.2144x; 1.2144x over previous
"""Trainium2 Bass kernel for a 2-layer tanh DeepRNN.

Problem: inputs [64, 1024, 256] fp32, two stacked RNN layers (H=512):
    h0_t = tanh(x_t @ W_xh0 + h0_{t-1} @ W_hh0 + b_h0)
    h1_t = tanh(h0_t @ W_xh1 + h1_{t-1} @ W_hh1 + b_h1)
Output: h1 sequence [64, 1024, 512] fp32.

Sharding: data-parallel over batch, 8 cores x B_local=8, weights replicated.

Schedule (v3): the per-step cost is dominated by LDWEIGHTS — each step each
layer reloads all 16 [128,128] W_hh tiles into the PE array for a tiny
(N=8) matmul.  So:
  * W_hh is stored fp8-e4m3 (pre-scaled x256 on the host; the 1/256 rescale
    rides the ACT engine's activation `scale` port), roughly halving/
    quartering the fast-weight-load time vs bf16.  Everything else stays
    bf16 with fp32 PSUM accumulation; only the recurrent weights are
    quantized.
  * The two layer recurrences are independent dependency chains (layer 1
    runs 2 chunks behind layer 0) interleaved at step granularity, so one
    chain's tanh round-trip hides behind the other chain's matmuls.
  * tanh is issued per 128-row output chunk (mc) straight after that
    chunk's 4 contraction matmuls, bias via the ACT bias port.
  * x arrives pre-transposed/pre-cast from the host; the output leaves in
    the transposed on-chip layout as bf16 and the host restores [B,T,H]
    fp32 — no on-device transposes at all.
"""

import sys

import numpy as np

sys.path.insert(0, "/opt/trn_rl_repo")

import ml_dtypes  # noqa: E402

import concourse.bacc as bacc  # noqa: E402
import concourse.tile as tile  # noqa: E402
from concourse import mybir  # noqa: E402
from concourse.bass_utils import run_bass_kernel_spmd  # noqa: E402

F32 = mybir.dt.float32
BF16 = mybir.dt.bfloat16
FP8 = mybir.dt.float8e4
Tanh = mybir.ActivationFunctionType.Tanh

B_FULL, T, I, H = 64, 1024, 256, 512
NCORES = 8
B = B_FULL // NCORES  # 8 rows per core
CH = 16               # timesteps per chunk (16*8 cols per mc = one PSUM bank)
NCH = T // CH
S0 = 64               # h0T ring slots (4 chunks: consumed up to 2 chunks late)
S1 = 32               # h1T ring slots

USE_FP8 = True        # fp8-e4m3 recurrent weights (x256 host pre-scale)
WSCALE = 256.0 if USE_FP8 else 1.0


def _mm(nc, out, lhsT, rhs, start, stop):
    nc.tensor.matmul(out, lhsT, rhs, start=start, stop=stop, skip_group_check=True)


def build_nc(nch=NCH, reps=1):
    nc = bacc.Bacc("TRN2", target_bir_lowering=False, debug=False)

    # x pre-transposed/pre-cast on host: [c, ic, i_rel, t, b] bf16
    x_d = nc.dram_tensor("x", [NCH, 2, 128, CH, B], BF16, kind="ExternalInput")
    wxh0_d = nc.dram_tensor("W_xh0", [I, H], F32, kind="ExternalInput")
    whh0_d = nc.dram_tensor("W_hh0", [H, H], FP8 if USE_FP8 else F32,
                            kind="ExternalInput")
    b0_d = nc.dram_tensor("b_h0", [H], F32, kind="ExternalInput")
    wxh1_d = nc.dram_tensor("W_xh1", [H, H], F32, kind="ExternalInput")
    whh1_d = nc.dram_tensor("W_hh1", [H, H], FP8 if USE_FP8 else F32,
                            kind="ExternalInput")
    b1_d = nc.dram_tensor("b_h1", [H], F32, kind="ExternalInput")
    # output in on-chip layout: [c, p(h_rel), slot*32+mc*8+b] bf16
    out_d = nc.dram_tensor("out", [NCH, 128, CH * 32], BF16, kind="ExternalOutput")

    with tile.TileContext(nc) as tc:
        _body(tc, nch, x_d, (wxh0_d, whh0_d, b0_d, wxh1_d, whh1_d, b1_d),
              out_d, reps=reps)
    nc.compile()
    return nc


def _body(tc, nch, x_d, w_d, out_d, reps=1):
    import contextlib

    nc = tc.nc
    wxh0_d, whh0_d, b0_d, wxh1_d, whh1_d, b1_d = w_d

    ctx = contextlib.ExitStack()
    with ctx:
        consts = ctx.enter_context(tc.tile_pool(name="consts", bufs=1))
        wstage = ctx.enter_context(tc.tile_pool(name="wstage", bufs=2))
        xpool = ctx.enter_context(tc.tile_pool(name="xpool", bufs=4))
        state = ctx.enter_context(tc.tile_pool(name="state", bufs=1))
        ps_l0 = ctx.enter_context(tc.tile_pool(name="ps_l0", bufs=2, space="PSUM"))
        ps_l1 = ctx.enter_context(tc.tile_pool(name="ps_l1", bufs=2, space="PSUM"))

        # ---- one-time constants ----
        def load_w_bf16(dram_ap, rows, name):
            # fp32 DRAM -> bf16 [128,128] tiles (one per (kc, mc) chunk so
            # every matmul's stationary operand is a whole tile at offset 0
            # -- keeps fast-weight-load eligibility unambiguous)
            tiles = []
            for kc in range(rows // 128):
                tmp = wstage.tile([128, H], F32, tag="wtmp")
                nc.sync.dma_start(tmp[:], dram_ap[kc * 128:(kc + 1) * 128, :])
                row = []
                for mc in range(4):
                    wt = consts.tile([128, 128], BF16, tag=f"{name}_{kc}_{mc}")
                    nc.vector.tensor_copy(wt[:], tmp[:, mc * 128:(mc + 1) * 128])
                    row.append(wt)
                tiles.append(row)
            return tiles

        def load_w_fp8(dram_ap, name):
            # fp8 DRAM (host pre-scaled/cast) -> direct [128,128] tile DMAs
            tiles = []
            for kc in range(4):
                row = []
                for mc in range(4):
                    wt = consts.tile([128, 128], FP8, tag=f"{name}_{kc}_{mc}")
                    nc.sync.dma_start(
                        wt[:], dram_ap[kc * 128:(kc + 1) * 128,
                                       mc * 128:(mc + 1) * 128])
                    row.append(wt)
                tiles.append(row)
            return tiles

        wxh0 = load_w_bf16(wxh0_d.ap(), I, "wxh0")   # [ic][hc] tiles
        wxh1 = load_w_bf16(wxh1_d.ap(), H, "wxh1")
        if USE_FP8:
            whh0 = load_w_fp8(whh0_d.ap(), "whh0")   # [kc][mc]
            whh1 = load_w_fp8(whh1_d.ap(), "whh1")
        else:
            whh0 = load_w_bf16(whh0_d.ap(), H, "whh0")
            whh1 = load_w_bf16(whh1_d.ap(), H, "whh1")

        def load_b(dram_ap, name):
            # [128, 4] f32: partition = h within chunk, free = mc (UNscaled --
            # ACT applies bias after the 1/WSCALE input rescale)
            bt = consts.tile([128, 4], F32, tag=name, name=name)
            nc.sync.dma_start(bt[:], dram_ap.rearrange("(m p) -> p m", p=128))
            return bt

        b0T = load_b(b0_d.ap(), "b0T")
        b1T = load_b(b1_d.ap(), "b1T")

        # ---- recurrent state rings: slots of [128, 32] (free = mc*8+b) ----
        h0T = state.tile([128, S0 * 32], BF16, tag="h0T")
        h1T = state.tile([128, S1 * 32], BF16, tag="h1T")
        nc.gpsimd.memset(h0T[:, (S0 - 1) * 32:S0 * 32], 0.0)  # h_{-1} = 0
        nc.gpsimd.memset(h1T[:, (S1 - 1) * 32:S1 * 32], 0.0)

        x_src = x_d.ap().rearrange("c ic p t b -> c p ic (t b)")  # [64,128,2,128]
        h0Tv = h0T[:].rearrange("p (s m b) -> p s m b", s=S0, b=B)

        ps0_t = {}
        ps1_t = {}
        x_t = {}
        inv = 1.0 / WSCALE

        # ---------- filler ops (emitted between recurrence steps) ----------
        def f_dma_x(c):
            def run():
                xT = xpool.tile([128, I], BF16, tag="xT", name="xT")
                nc.sync.dma_start(xT[:], x_src[c])
                x_t[c] = xT
            return run

        def f_xproj(c):
            # batched input projection for l0 chunk c into a fresh psum tile
            def mk():
                ps0_t[c] = ps_l0.tile([128, 512], F32, tag="ps0", name="ps0")
            ops = [mk]
            def mmop(hc, ic):
                def run():
                    xT = x_t[c]
                    _mm(nc, ps0_t[c][:, hc * 128:(hc + 1) * 128],
                        wxh0[ic][hc][:], xT[:, ic * 128:(ic + 1) * 128],
                        start=(hc == 0 and ic == 0), stop=False)
                return run
            for hc in range(4):
                for ic in range(2):
                    ops.append(mmop(hc, ic))
            def done():
                x_t.pop(c, None)
            ops.append(done)
            return ops

        def f_l1proj(c):
            # batched h0 projection for l1 chunk c into a fresh psum tile
            base = (c % 4) * CH
            def mk():
                ps1_t[c] = ps_l1.tile([128, 512], F32, tag="ps1", name="ps1")
            ops = [mk]
            def mmop(hc, kc):
                def run():
                    _mm(nc, ps1_t[c][:, hc * 128:(hc + 1) * 128],
                        wxh1[kc][hc][:], h0Tv[:, base:base + CH, kc],
                        start=(hc == 0 and kc == 0), stop=False)
                return run
            for hc in range(4):
                for kc in range(4):
                    ops.append(mmop(hc, kc))
            return ops

        # ---------- recurrence steps ----------
        def l0_step(c, tt):
            ps = ps0_t[c]
            s = (c % 4) * CH + tt
            prev = ((s - 1) % S0) * 32
            for mc in range(4):
                o = mc * 128 + tt * 8
                for kc in range(4):
                    _mm(nc, ps[:, o:o + 8], whh0[kc][mc][:],
                        h0T[:, prev + kc * 8:prev + kc * 8 + 8],
                        start=False, stop=(kc == 3))
                nc.scalar.activation(h0T[:, s * 32 + mc * 8:s * 32 + mc * 8 + 8],
                                     ps[:, o:o + 8], Tanh,
                                     bias=b0T[:, mc:mc + 1], scale=inv)
            if tt == CH - 1:
                ps0_t.pop(c)

        def l1_step(c, tt):
            ps = ps1_t[c]
            s = (c % 2) * CH + tt
            prev = ((s - 1) % S1) * 32
            for mc in range(4):
                o = mc * 128 + tt * 8
                for kc in range(4):
                    _mm(nc, ps[:, o:o + 8], whh1[kc][mc][:],
                        h1T[:, prev + kc * 8:prev + kc * 8 + 8],
                        start=False, stop=(kc == 3))
                nc.scalar.activation(h1T[:, s * 32 + mc * 8:s * 32 + mc * 8 + 8],
                                     ps[:, o:o + 8], Tanh,
                                     bias=b1T[:, mc:mc + 1], scale=inv)
            if tt == CH - 1:
                # stream the finished chunk (bf16, on-chip layout) to DRAM
                nc.sync.dma_start(out_d.ap()[c],
                                  h1T[:, (c % 2) * CH * 32:((c % 2) + 1) * CH * 32])
                ps1_t.pop(c)

        def main_loop():
            # prologue: get chunk 0 (and chunk 1's x) in flight
            for op in [f_dma_x(0), f_dma_x(1)] + f_xproj(0):
                op()
            for c in range(nch + 2):
                fillers = []
                if c + 2 < nch:
                    fillers.append(f_dma_x(c + 2))
                if c + 1 < nch:
                    fillers += f_xproj(c + 1)
                if 0 <= c - 1 < nch:
                    fillers += f_l1proj(c - 1)
                nf = len(fillers)
                fi = 0
                for tt in range(CH):
                    if c < nch:
                        l0_step(c, tt)
                    if c >= 2:
                        l1_step(c - 2, tt)
                    tgt = (nf * (tt + 1)) // CH
                    while fi < tgt:
                        fillers[fi]()
                        fi += 1

        if reps > 1:
            # timing mode: repeat the whole body on-device so the kernel time
            # dominates the (network-tunneled) host<->device transfer wall.
            with tc.For_i(0, reps, 1):
                main_loop()
        else:
            main_loop()


_NC_CACHE = {}


def _get_nc(nch=NCH):
    if nch not in _NC_CACHE:
        _NC_CACHE[nch] = build_nc(nch)
    return _NC_CACHE[nch]


def _prep_inputs(inputs):
    """Host-side transforms shared by kernel() and test harnesses."""
    x = np.asarray(inputs["inputs"], dtype=np.float32)
    s = np.float32(WSCALE)
    shared = {
        "W_xh0": np.asarray(inputs["W_xh0"], np.float32) * s,
        "b_h0": np.asarray(inputs["b_h0"], np.float32),
        "W_xh1": np.asarray(inputs["W_xh1"], np.float32) * s,
        "b_h1": np.asarray(inputs["b_h1"], np.float32),
    }
    for k in ("W_hh0", "W_hh1"):
        w = np.asarray(inputs[k], np.float32) * s
        if USE_FP8:
            shared[k] = w.astype(ml_dtypes.float8_e4m3)
        else:
            shared[k] = w
    in_maps = []
    for c in range(NCORES):
        xs = x[c * B:(c + 1) * B]                       # [B, T, I]
        xt = xs.reshape(B, NCH, CH, 2, 128).transpose(1, 3, 4, 2, 0)
        in_maps.append(dict(
            shared, x=np.ascontiguousarray(xt.astype(ml_dtypes.bfloat16))))
    return in_maps


def _post_output(res_list):
    outs = []
    for r in res_list:
        o = np.asarray(r["out"])                        # [NCH, 128, CH*32] bf16
        o = o.reshape(NCH, 128, CH, 4, B).transpose(4, 0, 2, 3, 1)
        outs.append(o.reshape(B, T, H).astype(np.float32))
    return np.concatenate(outs, axis=0)


def kernel(**inputs):
    in_maps = _prep_inputs(inputs)
    nc = _get_nc()
    res = run_bass_kernel_spmd(nc, in_maps, core_ids=list(range(NCORES)))
    return _post_output(res.results)


# revision 6
# speedup vs baseline: 2.7631x; 2.2752x over previous
"""Trainium2 Bass kernel for a 2-layer tanh DeepRNN.

Problem: inputs [64, 1024, 256] fp32, two stacked RNN layers (H=512):
    h0_t = tanh(x_t @ W_xh0 + h0_{t-1} @ W_hh0 + b_h0)
    h1_t = tanh(h0_t @ W_xh1 + h1_{t-1} @ W_hh1 + b_h1)
Output: h1 sequence [64, 1024, 512] fp32.

Sharding: data-parallel over batch, 8 cores x B_local=8, weights replicated.

Schedule (v3): the per-step cost is dominated by LDWEIGHTS — each step each
layer reloads all 16 [128,128] W_hh tiles into the PE array for a tiny
(N=8) matmul.  So:
  * W_hh is stored fp8-e4m3 (pre-scaled x256 on the host; the 1/256 rescale
    rides the ACT engine's activation `scale` port), roughly halving/
    quartering the fast-weight-load time vs bf16.  Everything else stays
    bf16 with fp32 PSUM accumulation; only the recurrent weights are
    quantized.
  * The two layer recurrences are independent dependency chains (layer 1
    runs 2 chunks behind layer 0) interleaved at step granularity, so one
    chain's tanh round-trip hides behind the other chain's matmuls.
  * tanh is issued per 128-row output chunk (mc) straight after that
    chunk's 4 contraction matmuls, bias via the ACT bias port.
  * x arrives pre-transposed/pre-cast from the host; the output leaves in
    the transposed on-chip layout as bf16 and the host restores [B,T,H]
    fp32 — no on-device transposes at all.
"""

import sys

import numpy as np

sys.path.insert(0, "/opt/trn_rl_repo")

import ml_dtypes  # noqa: E402

import concourse.bacc as bacc  # noqa: E402
import concourse.tile as tile  # noqa: E402
from concourse import mybir  # noqa: E402
from concourse.bass_utils import run_bass_kernel_spmd  # noqa: E402

F32 = mybir.dt.float32
BF16 = mybir.dt.bfloat16
FP8 = mybir.dt.float8e4
Tanh = mybir.ActivationFunctionType.Tanh

B_FULL, T, I, H = 64, 1024, 256, 512
NCORES = 8
B = B_FULL // NCORES  # 8 rows per core
CH = 16               # timesteps per chunk (16*8 cols per mc = one PSUM bank)
NCH = T // CH
S0 = 64               # h0T ring slots (4 chunks: consumed up to 2 chunks late)
S1 = 32               # h1T ring slots

USE_FP8 = True        # fp8-e4m3 recurrent weights (x256 host pre-scale)
WSCALE = 256.0 if USE_FP8 else 1.0


def _mm(nc, out, lhsT, rhs, start, stop):
    nc.tensor.matmul(out, lhsT, rhs, start=start, stop=stop, skip_group_check=True)


def build_nc(nch=NCH, reps=1):
    nc = bacc.Bacc("TRN2", target_bir_lowering=False, debug=False)

    # x pre-transposed/pre-cast on host: [c, ic, i_rel, t, b] bf16
    x_d = nc.dram_tensor("x", [NCH, 2, 128, CH, B], BF16, kind="ExternalInput")
    wxh0_d = nc.dram_tensor("W_xh0", [I, H], F32, kind="ExternalInput")
    whh0_d = nc.dram_tensor("W_hh0", [H, H], FP8 if USE_FP8 else F32,
                            kind="ExternalInput")
    b0_d = nc.dram_tensor("b_h0", [H], F32, kind="ExternalInput")
    wxh1_d = nc.dram_tensor("W_xh1", [H, H], F32, kind="ExternalInput")
    whh1_d = nc.dram_tensor("W_hh1", [H, H], FP8 if USE_FP8 else F32,
                            kind="ExternalInput")
    b1_d = nc.dram_tensor("b_h1", [H], F32, kind="ExternalInput")
    # output in on-chip layout: [c, p(h_rel), slot*32+mc*8+b] bf16
    out_d = nc.dram_tensor("out", [NCH, 128, CH * 32], BF16, kind="ExternalOutput")

    with tile.TileContext(nc) as tc:
        _body(tc, nch, x_d, (wxh0_d, whh0_d, b0_d, wxh1_d, whh1_d, b1_d),
              out_d, reps=reps)
    nc.compile()
    return nc


def _body(tc, nch, x_d, w_d, out_d, reps=1):
    import contextlib

    nc = tc.nc
    wxh0_d, whh0_d, b0_d, wxh1_d, whh1_d, b1_d = w_d

    ctx = contextlib.ExitStack()
    with ctx:
        consts = ctx.enter_context(tc.tile_pool(name="consts", bufs=1))
        wstage = ctx.enter_context(tc.tile_pool(name="wstage", bufs=2))
        xpool = ctx.enter_context(tc.tile_pool(name="xpool", bufs=4))
        state = ctx.enter_context(tc.tile_pool(name="state", bufs=1))
        ps_l0 = ctx.enter_context(tc.tile_pool(name="ps_l0", bufs=2, space="PSUM"))
        ps_l1 = ctx.enter_context(tc.tile_pool(name="ps_l1", bufs=2, space="PSUM"))

        # ---- one-time constants ----
        def load_w_bf16(dram_ap, rows, name):
            # fp32 DRAM -> bf16 [128,128] tiles (one per (kc, mc) chunk so
            # every matmul's stationary operand is a whole tile at offset 0
            # -- keeps fast-weight-load eligibility unambiguous)
            tiles = []
            for kc in range(rows // 128):
                tmp = wstage.tile([128, H], F32, tag="wtmp")
                nc.sync.dma_start(tmp[:], dram_ap[kc * 128:(kc + 1) * 128, :])
                row = []
                for mc in range(4):
                    wt = consts.tile([128, 128], BF16, tag=f"{name}_{kc}_{mc}")
                    nc.vector.tensor_copy(wt[:], tmp[:, mc * 128:(mc + 1) * 128])
                    row.append(wt)
                tiles.append(row)
            return tiles

        def load_w_fp8(dram_ap, name):
            # fp8 DRAM (host pre-scaled/cast) -> direct [128,128] tile DMAs
            tiles = []
            for kc in range(4):
                row = []
                for mc in range(4):
                    wt = consts.tile([128, 128], FP8, tag=f"{name}_{kc}_{mc}")
                    nc.sync.dma_start(
                        wt[:], dram_ap[kc * 128:(kc + 1) * 128,
                                       mc * 128:(mc + 1) * 128])
                    row.append(wt)
                tiles.append(row)
            return tiles

        wxh0 = load_w_bf16(wxh0_d.ap(), I, "wxh0")   # [ic][hc] tiles
        wxh1 = load_w_bf16(wxh1_d.ap(), H, "wxh1")
        if USE_FP8:
            whh0 = load_w_fp8(whh0_d.ap(), "whh0")   # [kc][mc]
            whh1 = load_w_fp8(whh1_d.ap(), "whh1")
        else:
            whh0 = load_w_bf16(whh0_d.ap(), H, "whh0")
            whh1 = load_w_bf16(whh1_d.ap(), H, "whh1")

        ones = consts.tile([1, 128], BF16, tag="ones")
        nc.gpsimd.memset(ones[:], 1.0)

        def load_b(dram_ap, name):
            # [1, 512] bf16 (host pre-scaled by WSCALE): enters the PSUM via
            # rank-1 (b x ones) matmuls in each chunk prologue
            tmp = wstage.tile([1, H], F32, tag="btmp")
            nc.sync.dma_start(tmp[:], dram_ap.unsqueeze(0))
            bt = consts.tile([1, H], BF16, tag=name, name=name)
            nc.vector.tensor_copy(bt[:], tmp[:])
            return bt

        b0T = load_b(b0_d.ap(), "b0T")
        b1T = load_b(b1_d.ap(), "b1T")

        # ---- recurrent state rings: slots of [128, 32] (free = mc*8+b) ----
        h0T = state.tile([128, S0 * 32], BF16, tag="h0T")
        h1T = state.tile([128, S1 * 32], BF16, tag="h1T")
        nc.gpsimd.memset(h0T[:, (S0 - 1) * 32:S0 * 32], 0.0)  # h_{-1} = 0
        nc.gpsimd.memset(h1T[:, (S1 - 1) * 32:S1 * 32], 0.0)

        x_src = x_d.ap().rearrange("c ic p t b -> c p ic (t b)")  # [64,128,2,128]
        h0Tv = h0T[:].rearrange("p (s m b) -> p s m b", s=S0, b=B)

        ps0_t = {}
        ps1_t = {}
        x_t = {}
        inv = 1.0 / WSCALE

        # ---------- filler ops (emitted between recurrence steps) ----------
        def f_dma_x(c):
            def run():
                xT = xpool.tile([128, I], BF16, tag="xT", name="xT")
                nc.sync.dma_start(xT[:], x_src[c])
                x_t[c] = xT
            return run

        def f_xproj(c):
            # batched input projection for l0 chunk c into a fresh psum tile
            def mk():
                ps0_t[c] = ps_l0.tile([128, 512], F32, tag="ps0", name="ps0")
            ops = [mk]
            def biasop(hc):
                def run():
                    _mm(nc, ps0_t[c][:, hc * 128:(hc + 1) * 128],
                        b0T[:, hc * 128:(hc + 1) * 128], ones[:],
                        start=(hc == 0), stop=False)
                return run
            for hc in range(4):
                ops.append(biasop(hc))
            def mmop(hc, ic):
                def run():
                    xT = x_t[c]
                    _mm(nc, ps0_t[c][:, hc * 128:(hc + 1) * 128],
                        wxh0[ic][hc][:], xT[:, ic * 128:(ic + 1) * 128],
                        start=False, stop=False)
                return run
            for hc in range(4):
                for ic in range(2):
                    ops.append(mmop(hc, ic))
            def done():
                x_t.pop(c, None)
            ops.append(done)
            return ops

        def f_l1proj(c):
            # batched h0 projection for l1 chunk c into a fresh psum tile
            base = (c % 4) * CH
            def mk():
                ps1_t[c] = ps_l1.tile([128, 512], F32, tag="ps1", name="ps1")
            ops = [mk]
            def biasop(hc):
                def run():
                    _mm(nc, ps1_t[c][:, hc * 128:(hc + 1) * 128],
                        b1T[:, hc * 128:(hc + 1) * 128], ones[:],
                        start=(hc == 0), stop=False)
                return run
            for hc in range(4):
                ops.append(biasop(hc))
            def mmop(hc, kc):
                def run():
                    _mm(nc, ps1_t[c][:, hc * 128:(hc + 1) * 128],
                        wxh1[kc][hc][:], h0Tv[:, base:base + CH, kc],
                        start=False, stop=False)
                return run
            for hc in range(4):
                for kc in range(4):
                    ops.append(mmop(hc, kc))
            return ops

        # ---------- recurrence steps ----------
        def l0_step(c, tt):
            ps = ps0_t[c]
            s = (c % 4) * CH + tt
            prev = ((s - 1) % S0) * 32
            for mc in range(4):
                o = mc * 128 + tt * 8
                for kc in range(4):
                    _mm(nc, ps[:, o:o + 8], whh0[kc][mc][:],
                        h0T[:, prev + kc * 8:prev + kc * 8 + 8],
                        start=False, stop=(kc == 3))
            nc.scalar.activation(
                h0T[:, s * 32:(s + 1) * 32].rearrange("p (m b) -> p m b", b=B),
                ps[:].rearrange("p (m t b) -> p m t b", m=4, t=CH)[:, :, tt],
                Tanh, scale=inv)
            if tt == CH - 1:
                ps0_t.pop(c)

        def l1_step(c, tt):
            ps = ps1_t[c]
            s = (c % 2) * CH + tt
            prev = ((s - 1) % S1) * 32
            for mc in range(4):
                o = mc * 128 + tt * 8
                for kc in range(4):
                    _mm(nc, ps[:, o:o + 8], whh1[kc][mc][:],
                        h1T[:, prev + kc * 8:prev + kc * 8 + 8],
                        start=False, stop=(kc == 3))
            nc.scalar.activation(
                h1T[:, s * 32:(s + 1) * 32].rearrange("p (m b) -> p m b", b=B),
                ps[:].rearrange("p (m t b) -> p m t b", m=4, t=CH)[:, :, tt],
                Tanh, scale=inv)
            if tt == CH - 1:
                # stream the finished chunk (bf16, on-chip layout) to DRAM
                nc.sync.dma_start(out_d.ap()[c],
                                  h1T[:, (c % 2) * CH * 32:((c % 2) + 1) * CH * 32])
                ps1_t.pop(c)

        def main_loop():
            # prologue: get chunk 0 (and chunk 1's x) in flight
            for op in [f_dma_x(0), f_dma_x(1)] + f_xproj(0):
                op()
            for c in range(nch + 2):
                fillers = []
                if c + 2 < nch:
                    fillers.append(f_dma_x(c + 2))
                if c + 1 < nch:
                    fillers += f_xproj(c + 1)
                if 0 <= c - 1 < nch:
                    fillers += f_l1proj(c - 1)
                nf = len(fillers)
                fi = 0
                for tt in range(CH):
                    if c < nch:
                        l0_step(c, tt)
                    tgt = (nf * (tt + 1)) // CH
                    while fi < tgt:
                        fillers[fi]()
                        fi += 1
                    if c >= 2:
                        l1_step(c - 2, tt)

        if reps > 1:
            # timing mode: repeat the whole body on-device so the kernel time
            # dominates the (network-tunneled) host<->device transfer wall.
            with tc.For_i(0, reps, 1):
                main_loop()
        else:
            main_loop()


_NC_CACHE = {}


def _get_nc(nch=NCH):
    if nch not in _NC_CACHE:
        _NC_CACHE[nch] = build_nc(nch)
    return _NC_CACHE[nch]


def _prep_inputs(inputs):
    """Host-side transforms shared by kernel() and test harnesses."""
    x = np.asarray(inputs["inputs"], dtype=np.float32)
    s = np.float32(WSCALE)
    shared = {
        "W_xh0": np.asarray(inputs["W_xh0"], np.float32) * s,
        "b_h0": np.asarray(inputs["b_h0"], np.float32) * s,
        "W_xh1": np.asarray(inputs["W_xh1"], np.float32) * s,
        "b_h1": np.asarray(inputs["b_h1"], np.float32) * s,
    }
    for k in ("W_hh0", "W_hh1"):
        w = np.asarray(inputs[k], np.float32) * s
        if USE_FP8:
            shared[k] = w.astype(ml_dtypes.float8_e4m3)
        else:
            shared[k] = w
    in_maps = []
    for c in range(NCORES):
        xs = x[c * B:(c + 1) * B]                       # [B, T, I]
        xt = xs.reshape(B, NCH, CH, 2, 128).transpose(1, 3, 4, 2, 0)
        in_maps.append(dict(
            shared, x=np.ascontiguousarray(xt.astype(ml_dtypes.bfloat16))))
    return in_maps


def _post_output(res_list):
    outs = []
    for r in res_list:
        o = np.asarray(r["out"])                        # [NCH, 128, CH*32] bf16
        o = o.reshape(NCH, 128, CH, 4, B).transpose(4, 0, 2, 3, 1)
        outs.append(o.reshape(B, T, H).astype(np.float32))
    return np.concatenate(outs, axis=0)


def kernel(**inputs):
    in_maps = _prep_inputs(inputs)
    nc = _get_nc()
    res = run_bass_kernel_spmd(nc, in_maps, core_ids=list(range(NCORES)))
    return _post_output(res.results)
